# revision 2
# baseline (speedup 1.0000x reference)
"""DIN attention layer kernel for Trainium2 — v2 (bf16, rolling-128 packing).

Math (per batch):
  x  = concat([q, ub, q-ub, q*ub], -1)             # [T,144]
  h1 = sigmoid(x @ W1 + b1)                        # [T,80]
  h2 = sigmoid(h1 @ W2 + b2)                       # [T,40]
  s  = h2 @ W3 + b3                                # [T,1]
  w  = softmax(s.T * mask)                         # [1,T] (multiplicative mask)
  out = w @ ub                                     # [1,36]

Device-side structure (per core, bc=512 batches, data-parallel over 8 cores):
  * mm1 fold (host): x@W1 = ubaug @ waug_b with ubaug=[ub,1] (K=37) and
    per-batch waug_b=[(Wb-Wc)+diag(q)Wd ; q(Wa+Wc)+b1] ([37,80]).
  * sigmoid -> 0.5+0.5*tanh(x/2); the affine is folded into下 next-layer
    weights, so the device only evaluates tanh (shares an ACT table with exp).
  * rolling-128 packing: h1 units of a group of 8 batches (8*80=640=5*128)
    are packed 128-per-window; host builds block-diagonal stacked stationaries
    (K in {74,111}) and vertically-stacked ubaug^T streams, so every matmul
    and tanh runs with all 128 partitions active.  Same for h2 (40*16=640
    units per 16-batch block = 5 windows) with shared W2 sliced into
    block-diagonal pieces, and for the score layer with W3 pieces that place
    s directly into a [64 batch, 200 t] PSUM tile.
  * softmax on [64,200]; weights transposed on-chip via PE transpose; final
    weighted sum uses per-batch [128,1] stationaries against a host-packed
    zero-padded ub window layout, accumulating straight in PSUM; results DMA
    directly PSUM->DRAM.
All matmul operands bf16 (PSUM accumulation fp32); fp32 streams cost 4
cycles/col on the PE, bf16 costs 1.
"""

from contextlib import ExitStack

import numpy as np
import ml_dtypes

import concourse.bass as bass
import concourse.bacc as bacc
import concourse.tile as tile
from concourse import mybir
from concourse.bass_utils import run_bass_kernel_spmd

B, T, E = 4096, 200, 36
MM2_INTERLEAVE = False  # interleaved accum groups within a PSUM bank corrupt results on HW
MM3_REORDER = True  # needs h2p bufs >= 20
PH0_SPLIT = True
N_CORES = 8
BC = B // N_CORES            # 512 batches per core
PH_N = 8                     # phases per core
PB = BC // PH_N              # 64 batches per phase
F32 = mybir.dt.float32
BF16 = mybir.dt.bfloat16
AF = mybir.ActivationFunctionType
F8D = mybir.dt.float8e4
BF = ml_dtypes.bfloat16
F8 = ml_dtypes.float8_e4m3

# ---- rolling-128 index structure (group = 8 batches, block = 16) ----

# mm1: h1-units 8*80=640 = 5 windows of 128
MM1_BSETS = []               # per j: ordered local batches in window
for j in range(5):
    bs = sorted({u // 80 for u in range(128 * j, 128 * j + 128)})
    MM1_BSETS.append(bs)
MM1_K = [37 * len(bs) for bs in MM1_BSETS]          # [74, 111, 74, 111, 74]

# mm2: h2-units 16*40=640 = 5 windows (wt); h1-units 16*80=1280 = 10 tiles
def _mm2_pieces():
    """per wt: list of (j, fat, mhi); fat piece listed first."""
    out = []
    for wt in range(5):
        bf_, bl_ = (128 * wt) // 40, (128 * wt + 127) // 40
        r0, r1 = 80 * bf_, 80 * (bl_ + 1)
        js = list(range(r0 // 128, (r1 - 1) // 128 + 1))
        pieces = []
        for j in js:
            p1lo, p1hi = max(0, r0 - 128 * j), min(128, r1 - 128 * j)
            # h2-cols (within window wt) whose batch overlaps tile j rows
            mhi = 0
            for p2 in range(128):
                b2 = (128 * wt + p2) // 40
                if 80 * b2 < 128 * (j + 1) and 80 * (b2 + 1) > 128 * j:
                    mhi = p2 + 1
            pieces.append((j, p1hi - p1lo, mhi))
        # fat piece = largest row overlap; it goes first with start=True, M=128
        fat = max(range(len(pieces)), key=lambda i: pieces[i][1])
        order = [fat] + [i for i in range(len(pieces)) if i != fat]
        out.append([(pieces[i][0], i == fat, 128 if i == fat else pieces[i][2])
                    for i in order])
    return out

MM2_PIECES = _mm2_pieces()

# mm3: batches per h2-window wt (always 4 consecutive, first = bf)
MM3_BF = [(128 * wt) // 40 for wt in range(5)]


def _build_w2cat(w2f):
    """[128, sum(M)] block-diagonal W2 pieces + per-piece col offsets."""
    cols = []
    offs = []
    off = 0
    for wt in range(5):
        lst = []
        for (j, _fat, M) in MM2_PIECES[wt]:
            pc = np.zeros((128, M), dtype=np.float32)
            for p1 in range(128):
                u1 = 128 * j + p1
                b1_ = u1 // 80
                for p2 in range(M):
                    u2 = 128 * wt + p2
                    if u2 // 40 == b1_:
                        pc[p1, p2] = w2f[u1 % 80, u2 % 40]
            cols.append(pc)
            lst.append((off, M))
            off += M
        offs.append(lst)
    return np.concatenate(cols, axis=1), offs


def _build_w3cat(w3f):
    """[128, sum(M')] score pieces for (blk parity, wt) + offsets.

    Piece (par, wt): column m <-> batch 32*(blk//2)+m of the phase; row p
    (h2-unit 128*wt+p, local batch lb=(128wt+p)//40) maps to m = 16*par+lb.
    M' = 16*par + bf(wt) + 4 <= 32, so out base partition 32*(blk//2) is
    32-aligned."""
    cols = []
    offs = {}
    off = 0
    for par in range(2):
        for wt in range(5):
            Mp = 16 * par + MM3_BF[wt] + 4
            pc = np.zeros((128, Mp), dtype=np.float32)
            for p in range(128):
                u2 = 128 * wt + p
                pc[p, 16 * par + u2 // 40] = w3f[u2 % 40]
            cols.append(pc)
            offs[(par, wt)] = (off, Mp)
            off += Mp
    return np.concatenate(cols, axis=1), offs


def build_module():
    nc = bacc.Bacc(
        "TRN2", target_bir_lowering=False, debug=False,
        enable_asserts=False, num_devices=N_CORES,
    )

    # stat/ubj padded to 128 partitions (zero rows beyond K_j): one DMA call
    # per phase, spanning all 128 partitions, engages all 16 SDMA engines and
    # keeps the issuing sequencer's DIRECT2D count minimal.
    stat_d = nc.dram_tensor("stat", [128, 5, 64, 128], F8D,
                            kind="ExternalInput").ap()
    ubj_d = nc.dram_tensor("ubj", [128, 5, 64, 200], F8D,
                           kind="ExternalInput").ap()
    ubw_d = nc.dram_tensor("ubw", [128, BC, 2, 36], BF16,
                           kind="ExternalInput").ap()
    lens_d = nc.dram_tensor("lens", [BC, 1], F32, kind="ExternalInput").ap()
    w2cat_d = nc.dram_tensor("w2cat", [128, W2CAT_COLS], F8D,
                             kind="ExternalInput").ap()
    w3cat_d = nc.dram_tensor("w3cat", [128, W3CAT_COLS], F8D,
                             kind="ExternalInput").ap()
    b2roll_d = nc.dram_tensor("b2roll", [128, 5], F32,
                              kind="ExternalInput").ap()
    b3c_d = nc.dram_tensor("b3c", [64, 1], F32, kind="ExternalInput").ap()
    out_d = nc.dram_tensor("out", [BC, 36], F32, kind="ExternalOutput").ap()

    iota_d = nc.inline_tensor(
        np.broadcast_to(np.arange(200, dtype=np.float32), (64, 200)).copy(),
        name="iotat").ap()
    ident_d = nc.inline_tensor(np.eye(64, dtype=np.float32), name="ident").ap()

    with tile.TileContext(nc) as tc, ExitStack() as es:
        cpool = es.enter_context(tc.tile_pool(name="consts", bufs=1))
        statp = es.enter_context(tc.tile_pool(name="statp", bufs=3))
        ubjp = es.enter_context(tc.tile_pool(name="ubjp", bufs=3))
        ubwp = es.enter_context(tc.tile_pool(name="ubwp", bufs=3))
        lenp = es.enter_context(tc.tile_pool(name="lenp", bufs=3))
        h1p = es.enter_context(tc.tile_pool(name="h1p", bufs=9))
        h2p = es.enter_context(tc.tile_pool(name="h2p", bufs=24))
        smp = es.enter_context(tc.tile_pool(name="smp", bufs=2))
        wcp = es.enter_context(tc.tile_pool(name="wcp", bufs=2))
        m1p = es.enter_context(tc.tile_pool(name="m1p", bufs=2, space="PSUM"))
        m2p = es.enter_context(tc.tile_pool(name="m2p", bufs=3, space="PSUM"))
        sp = es.enter_context(tc.tile_pool(name="sp", bufs=2, space="PSUM"))
        wsp = es.enter_context(tc.tile_pool(name="wsp", bufs=1, space="PSUM"))

        iota_t = cpool.tile([64, 200], F32)
        nc.scalar.dma_start(out=iota_t, in_=iota_d)
        ident_t = cpool.tile([64, 64], F32)
        nc.scalar.dma_start(out=ident_t, in_=ident_d)
        w2cat_t = cpool.tile([128, W2CAT_COLS], F8D)
        nc.scalar.dma_start(out=w2cat_t, in_=w2cat_d)
        w3cat_t = cpool.tile([128, W3CAT_COLS], F8D)
        nc.scalar.dma_start(out=w3cat_t, in_=w3cat_d)
        b2roll_t = cpool.tile([128, 5], F32)
        nc.scalar.dma_start(out=b2roll_t, in_=b2roll_d)
        b3c_t = cpool.tile([64, 1], F32)
        nc.scalar.dma_start(out=b3c_t, in_=b3c_d)

        def emit_wsum(wph, wcols, ubw_t):
            """out[b] = sum_t w[b,t] ub[b,t,:] for phase wph."""
            for h in range(2):
                ws = wsp.tile([97, 2, 256], F32, tag="ws", name=f"ws{wph}_{h}")
                for b32 in range(32):
                    b = 32 * h + b32
                    pp, blk8 = b32 % 4, b32 // 4
                    col0 = 256 * (blk8 // 4) + 37 * (blk8 % 4)
                    for c in range(2):
                        nc.tensor.matmul(
                            ws[32 * pp:32 * pp + 1, col0 // 256,
                               col0 % 256:col0 % 256 + 36],
                            wcols[:, c, b:b + 1],
                            ubw_t[:, b, c, :],
                            start=(c == 0), stop=(c == 1),
                            tile_position=(0, 32 * pp))
                wsb = smp.tile([97, 2, 148], F32, tag="wsb", name=f"wb{wph}_{h}")
                nc.vector.tensor_copy(out=wsb, in_=ws[0:97, :, 0:148])
                nc.scalar.dma_start(
                    out=bass.AP(
                        tensor=out_d.tensor,
                        offset=out_d.offset + 36 * (PB * wph + 32 * h),
                        ap=[[36, 4], [576, 2], [144, 4], [1, 36]]),
                    in_=bass.AP(
                        tensor=wsb.tensor, offset=wsb.offset,
                        ap=[[296 * 32, 4], [148, 2], [37, 4], [1, 36]]))

        prev = None  # deferred weighted-sum state from previous phase
        for ph in range(PH_N):
            # ---- phase input loads: few fat 128-partition DMA calls, issue
            #      split across both HWDGE rings (sync + scalar) ----
            statT = statp.tile([128, 5, 8, 128], F8D, tag="stat",
                               name=f"st{ph}")
            ubjT = ubjp.tile([128, 5, 8, 200], F8D, tag="ubj", name=f"ub{ph}")
            if ph == 0 and PH0_SPLIT:
                # first phase: load per-2-groups so mm1 starts ASAP
                for q in range(4):
                    gsl = slice(2 * q, 2 * q + 2)
                    nc.sync.dma_start(out=statT[:, :, gsl, :],
                                      in_=stat_d[:, :, gsl, :])
                    nc.scalar.dma_start(out=ubjT[:, :, gsl, :],
                                        in_=ubj_d[:, :, gsl, :])
            else:
                nc.sync.dma_start(out=statT[0:64],
                                  in_=stat_d[0:64, :, 8 * ph:8 * ph + 8, :])
                nc.scalar.dma_start(out=statT[64:128],
                                    in_=stat_d[64:128, :, 8 * ph:8 * ph + 8, :])
                nc.sync.dma_start(out=ubjT[0:64],
                                  in_=ubj_d[0:64, :, 8 * ph:8 * ph + 8, :])
                nc.scalar.dma_start(out=ubjT[64:128],
                                    in_=ubj_d[64:128, :, 8 * ph:8 * ph + 8, :])
            # ubw/lens are only needed at the end of the phase; issue after
            # the stat/ubj loads so they don't delay mm1 in the queue
            ubw_t = ubwp.tile([128, PB, 2, 36], BF16, tag="ubw", name=f"uw{ph}")
            nc.sync.dma_start(out=ubw_t,
                              in_=ubw_d[:, PB * ph:PB * ph + PB, :, :])
            lencol = lenp.tile([64, 1], F32, tag="len", name=f"ln{ph}")
            nc.scalar.dma_start(out=lencol, in_=lens_d[PB * ph:PB * ph + PB, :])

            # ---- mm1 + tanh: h1 in rolling-128 layout ----
            h1map = {}   # global h1-tile index (0..39) -> (tile, chunk)
            for g in range(8):
                for pair in range(3):
                    js = [2 * pair, 2 * pair + 1] if pair < 2 else [4]
                    m1t = m1p.tile([128, 2, 256], F32, tag="m1",
                                   name=f"m1_{ph}_{g}_{pair}")
                    for ci, j in enumerate(js):
                        nc.tensor.matmul(
                            m1t[:, ci, 0:200],
                            statT[:, j, g, :], ubjT[:, j, g, :],
                            start=True, stop=True)
                    h1t = h1p.tile([128, len(js), 200], F8D, tag="h1",
                                   name=f"h1_{ph}_{g}_{pair}")
                    nc.scalar.activation(
                        out=h1t, in_=m1t[:, 0:len(js), 0:200],
                        func=AF.Tanh, scale=0.5)
                    for ci, j in enumerate(js):
                        h1map[5 * g + j] = (h1t, ci)

            # ---- mm2 + tanh: h2 rolling, per 16-batch block.  Pieces of
            #      adjacent windows are interleaved so back-to-back matmuls
            #      never accumulate into the same PSUM region (RAW stall). ----
            h2map = {}   # (blk, wt) -> tile
            for blk in range(4):
                m2t_of = {}
                for wt in range(5):
                    if wt % 2 == 0:
                        m2t = m2p.tile([128, 2, 256], F32, tag="m2",
                                       name=f"m2_{ph}_{blk}_{wt}")
                    m2t_of[wt] = (m2t, wt % 2)
                maxp = max(len(MM2_PIECES[wt]) for wt in range(5))
                order = ([(i, wt) for i in range(maxp) for wt in range(5)]
                         if MM2_INTERLEAVE else
                         [(i, wt) for wt in range(5) for i in range(maxp)])
                for i, wt in order:
                    if True:
                        if i >= len(MM2_PIECES[wt]):
                            continue
                        (j, fat, M), (off, _M2) = MM2_PIECES[wt][i], W2OFFS[wt][i]
                        h1t_, hci = h1map[10 * blk + j]
                        t_, ci = m2t_of[wt]
                        nc.tensor.matmul(
                            t_[0:M, ci, 0:200],
                            w2cat_t[:, off:off + M],
                            h1t_[:, hci, :],
                            start=fat, stop=True, skip_group_check=True)
                for wt in range(5):
                    t_, ci = m2t_of[wt]
                    h2t = h2p.tile([128, 200], F8D, tag="h2",
                                   name=f"h2_{ph}_{blk}_{wt}")
                    nc.scalar.activation(
                        out=h2t, in_=t_[:, ci, 0:200],
                        func=AF.Tanh, bias=b2roll_t[:, wt:wt + 1], scale=0.5)
                    h2map[(blk, wt)] = h2t

            # ---- mm3: scores straight into [64 batch, 200 t] PSUM ----
            s_ps = sp.tile([128, 256], F32, tag="s", name=f"s_{ph}")
            nc.vector.memset(s_ps[0:64, 0:200], 0.0)
            mm3_order = ([(wt, blk) for wt in range(5) for blk in (0, 2, 1, 3)]
                         if MM3_REORDER else
                         [(wt, blk) for blk in range(4) for wt in range(5)])
            for wt, blk in mm3_order:
                if True:
                    off, Mp = W3OFFS[(blk % 2, wt)]
                    p0 = 32 * (blk // 2)
                    nc.tensor.matmul(
                        s_ps[p0:p0 + Mp, 0:200],
                        w3cat_t[:, off:off + Mp],
                        h2map[(blk, wt)],
                        start=False, stop=(wt == 4 and blk == 3),
                        skip_group_check=True)

            # ---- weighted sum of phase ph-1 (fills the PE pipe while this
            #      phase's softmax runs on DVE/ACT) ----
            if prev is not None:
                emit_wsum(*prev)
            # ---- masked softmax over t ----
            mask_t = smp.tile([64, 200], F32, tag="mask", name=f"mk{ph}")
            nc.vector.tensor_scalar(
                out=mask_t, in0=iota_t, scalar1=lencol, scalar2=None,
                op0=mybir.AluOpType.is_lt)
            sb3 = smp.tile([64, 200], F32, tag="sb3", name=f"sb{ph}")
            nc.vector.tensor_scalar_add(sb3, s_ps[0:64, 0:200], b3c_t)
            masked = smp.tile([64, 200], F32, tag="masked", name=f"msk{ph}")
            nc.vector.tensor_mul(masked, sb3, mask_t)
            negmax = smp.tile([64, 1], F32, tag="negmax", name=f"nm{ph}")
            nc.vector.tensor_reduce(
                out=negmax, in_=masked, axis=mybir.AxisListType.X,
                op=mybir.AluOpType.max, negate=True)
            ew = smp.tile([64, 200], F32, tag="ew", name=f"ew{ph}")
            sumexp = smp.tile([64, 1], F32, tag="sumexp", name=f"se{ph}")
            nc.scalar.activation(
                out=ew, in_=masked, func=AF.Exp,
                bias=negmax, accum_out=sumexp)
            rz = smp.tile([64, 1], F32, tag="rz", name=f"rz{ph}")
            nc.vector.reciprocal(rz, sumexp)
            w_t = smp.tile([64, 200], F32, tag="wt", name=f"wt{ph}")
            nc.vector.tensor_scalar_mul(w_t, ew, rz)

            # ---- transpose w on-chip: [64,200] -> [128|72, 64] ----
            wT_ps = sp.tile([128, 256], F32, tag="s", name=f"wT_{ph}")
            nc.tensor.transpose(wT_ps[0:128, 0:64], w_t[:, 0:128], ident_t)
            nc.tensor.transpose(wT_ps[0:72, 64:128], w_t[:, 128:200], ident_t)
            wcols = wcp.tile([128, 2, 64], BF16, tag="wcols", name=f"wc{ph}")
            nc.vector.memset(wcols, 0.0)
            nc.vector.tensor_copy(out=wcols[:, 0, :], in_=wT_ps[0:128, 0:64])
            nc.vector.tensor_copy(out=wcols[0:72, 1, :],
                                  in_=wT_ps[0:72, 64:128])

            prev = (ph, wcols, ubw_t)

        emit_wsum(*prev)  # drain the last phase

    nc.compile()
    return nc


def host_prep(query_ad, user_behavior, user_behavior_length,
              W1, b1, W2, b2, W3, b3):
    q = np.asarray(query_ad, dtype=np.float32)
    ub = np.asarray(user_behavior, dtype=np.float32)
    lens = np.asarray(user_behavior_length)
    W1 = np.asarray(W1, dtype=np.float32)
    b1 = np.asarray(b1, dtype=np.float32)
    W2 = np.asarray(W2, dtype=np.float32)
    b2 = np.asarray(b2, dtype=np.float32)
    W3 = np.asarray(W3, dtype=np.float32)
    b3 = np.asarray(b3, dtype=np.float32)

    Wa, Wb, Wc, Wd = W1[0:36], W1[36:72], W1[72:108], W1[108:144]
    # per-batch mm1 weights [B, 37, 80]
    waug = np.empty((B, 37, 80), dtype=np.float32)
    waug[:, 0:36, :] = (Wb - Wc)[None] + q[:, :, None] * Wd[None]
    waug[:, 36, :] = q @ (Wa + Wc) + b1[None, :]

    # sigmoid->tanh folds
    w2f = 0.5 * W2
    b2f = 0.5 * (b2 + 0.5 * W2.sum(axis=0))
    w3f = 0.5 * W3[:, 0]
    b3c = float(b3[0] + 0.5 * W3.sum())

    w2cat, _ = _build_w2cat(w2f)
    w3cat, _ = _build_w3cat(w3f)
    b2roll = np.empty((128, 5), dtype=np.float32)
    for wt in range(5):
        b2roll[:, wt] = b2f[(128 * wt + np.arange(128)) % 40]

    # ubaug^T [B, 37, 200]
    ubaugT = np.empty((B, 37, 200), dtype=np.float32)
    ubaugT[:, 0:36, :] = ub.transpose(0, 2, 1)
    ubaugT[:, 36, :] = 1.0

    in_maps = []
    for c in range(N_CORES):
        sl = slice(BC * c, BC * (c + 1))
        waug_c = waug[sl].reshape(64, 8, 37, 80)
        ubT_c = ubaugT[sl].reshape(64, 8, 37, 200)
        ub_c = ub[sl]
        m = {}
        stp = np.zeros((128, 5, 64, 128), dtype=np.float32)
        ubp = np.zeros((128, 5, 64, 200), dtype=np.float32)
        for j in range(5):
            bs = MM1_BSETS[j]
            for idx, bl in enumerate(bs):
                ms = [mm for mm in range(128) if (128 * j + mm) // 80 == bl]
                mlo, mhi = ms[0], ms[-1] + 1
                klo = (128 * j + mlo) % 80
                stp[37 * idx:37 * idx + 37, j, :, mlo:mhi] = \
                    waug_c[:, bl, :, klo:klo + mhi - mlo].transpose(1, 0, 2)
                ubp[37 * idx:37 * idx + 37, j] = ubT_c[:, bl].transpose(1, 0, 2)
        m["stat"] = stp.astype(F8)
        m["ubj"] = ubp.astype(F8)
        ubwp_ = np.zeros((128, BC, 2, 36), dtype=np.float32)
        ubwp_[:, :, 0, :] = ub_c[:, 0:128, :].transpose(1, 0, 2)
        ubwp_[0:72, :, 1, :] = ub_c[:, 128:200, :].transpose(1, 0, 2)
        m["ubw"] = ubwp_.astype(BF)
        m["lens"] = lens[sl].astype(np.float32).reshape(BC, 1)
        m["w2cat"] = w2cat.astype(F8)
        m["w3cat"] = w3cat.astype(F8)
        m["b2roll"] = b2roll
        m["b3c"] = np.full((64, 1), b3c, dtype=np.float32)
        in_maps.append(m)
    return in_maps


# piece offset tables (geometry only; filled at import)
_w2c, W2OFFS = _build_w2cat(np.zeros((80, 40), dtype=np.float32))
W2CAT_COLS = _w2c.shape[1]
_w3c, W3OFFS = _build_w3cat(np.zeros((40,), dtype=np.float32))
W3CAT_COLS = _w3c.shape[1]

_NC_CACHE = {}


def get_module():
    if "nc" not in _NC_CACHE:
        _NC_CACHE["nc"] = build_module()
    return _NC_CACHE["nc"]


def kernel(query_ad, user_behavior, user_behavior_length,
           W1, b1, W2, b2, W3, b3, trace=False):
    nc = get_module()
    in_maps = host_prep(query_ad, user_behavior, user_behavior_length,
                        W1, b1, W2, b2, W3, b3)
    res = run_bass_kernel_spmd(nc, in_maps, core_ids=list(range(N_CORES)),
                               trace=trace)
    outs = [res.results[c]["out"] for c in range(N_CORES)]
    full = np.concatenate(outs, axis=0).reshape(B, 1, 36)
    if trace:
        kernel.last_result = res
    return full
